# revision 45
# baseline (speedup 1.0000x reference)
"""CNN+RNN fused Trainium2 kernel, 8-core data parallel (batch 8192 -> 1024/core).

Model: Conv2d(1->16, 3x3, pad=1)+bias+ReLU -> MaxPool2d(2) -> flatten ->
Linear(3136->256)+b_in -> r=relu(E0) -> 9x r=relu(r@W + b_in + r) ->
Linear(256->10)+b_out.

v2 design, 118.3us (TimelineSim) vs 135.3us v1 baseline; rel err 3.7e-3:
- Conv-as-matmul (A [112, 896] bf16, j-major class layout) over 14 pixel-row
  blocks; 56 drain chains (unit x batch-half), each reducing 4 pool-class
  psums [112,512] with a balanced mix of schemes: gamma chains use 2 ACT
  relu+bias leaves + 2 DVE scalar_tensor_tensor (fused bias+max) + 1 bf16
  TT max; beta chains (12 of 56, spread (ci%14) in {4,8,13}) use 4 ACT
  leaves + 3 TT, balancing ACT (~97us) vs DVE (~98.5us) busy.
- Three-stage software pipeline per chain over a 4-slot psum ring keeps 4
  conv banks + 4 W_in banks within the 8 PSUM banks while the PE runs one
  chain ahead of the drains.
- W_in (3136->256) accumulates in 4 x [128,512] psums, matmuls interleaved
  into the conv emission (SK=5 chains of skew) to fill PE idle; the tail is
  split per batch-half so r0 drains and the recurrence start early.
- Recurrence r' = relu(r @ (W+I) + b_in) runs in float32r: 1 PE cycle/row
  at N=512 (vs 4 for fp32) with ~1.2e-4 relative rounding - so r needs no
  bf16 shadow copy and each step drains with a SINGLE op per (mch, half)
  (ACT relu+bias or DVE tensor_scalar add/max, alternating). W+I fold
  removes the separate +r term. Recurrence psums reuse the freed W_in bank
  rings; W_out (f32r) is emitted per batch-half inside the last step.
- f32r DMAs ride the Pool/SWDGE queue: on the sync/HWDGE queue they corrupt
  later bf16 transfers (observed on HW). PE warmup: 8 short matmuls on a
  memset tile during the initial DMA window.
Engine busy (TimelineSim): DVE 98.5us, ACT 97.0us, PE 88.6us, span 118.3us.
"""
import sys
sys.path.insert(0, "/opt/trn_rl_repo")
from contextlib import ExitStack

import numpy as np
import ml_dtypes

import concourse.bacc as bacc
import concourse.tile as tile
from concourse import mybir
from concourse.bass_utils import run_bass_kernel_spmd

BF16 = ml_dtypes.bfloat16
NCORES = 8
B = 8192
BS = B // NCORES          # 1024 per core
C = 16
H = 256
OUT = 10
IMG = 28
NBLK = 14                 # pooled-row blocks (s)
HALO = 112                # 4 image rows
NPOOL = 28                # pooled K-tiles of 112 rows (28*112 = 3136)
NSTEP = 9
BETA_SET = (4, 8, 13)
REC_NQ = 2          # recurrence batch-chain count (2 or 4)
WSKEW = 5                 # chains of lag before W_in matmuls for a pooled tile-half

_CACHE = {}


def _build_amat(conv_w):
    """A [112, 896]: conv-as-matmul for one pooled-row block.

    Column m = mc*112 + q, mc = cls*2 + j, cls = a*2 + b (a=row-in-pair,
    b=col parity), channel c = 8j + q//14, pooled col jp = q%14.
    Input rows = halo pixels (4 image rows, row-major).
    """
    A = np.zeros((112, 8 * 112), np.float32)
    for mc in range(8):
        cls, j = mc // 2, mc % 2
        a, bpar = cls // 2, cls % 2
        for q in range(112):
            c = 8 * j + q // 14
            jp = q % 14
            m = mc * 112 + q
            cc = 2 * jp + bpar
            for di in range(3):
                for dj in range(3):
                    icol = cc - 1 + dj
                    if 0 <= icol < IMG:
                        A[(a + di) * IMG + icol, m] += conv_w[c, 0, di, dj]
    return A


def _build_wg(W_in):
    """Wg [112, 28*256]: W_in^T blocked to match pooled-tile layout.

    Pooled tile t = 2s+j holds rows q -> (c = 8j + q//14, i'=s, jp = q%14),
    i.e. W_in column c*196 + s*14 + jp.
    """
    Wg = np.zeros((112, NPOOL * H), np.float32)
    q = np.arange(112)
    for t in range(NPOOL):
        s, j = t // 2, t % 2
        cols = (8 * j + q // 14) * 196 + s * 14 + (q % 14)
        Wg[:, t * H:(t + 1) * H] = W_in[:, cols].T
    return Wg


def _build_graph():
    nc = bacc.Bacc("TRN2", target_bir_lowering=False, debug=False)
    f32, bf16 = mybir.dt.float32, mybir.dt.bfloat16
    f32r = mybir.dt.float32r
    AL = mybir.AluOpType
    RELU = mybir.ActivationFunctionType.Relu

    xt = nc.dram_tensor("xt", [840, BS], bf16, kind="ExternalInput").ap()
    amat = nc.dram_tensor("amat", [HALO, 896], bf16, kind="ExternalInput").ap()
    wg = nc.dram_tensor("wg", [112, NPOOL * H], bf16, kind="ExternalInput").ap()
    wrec = nc.dram_tensor("wrec", [128, 512], f32r, kind="ExternalInput").ap()
    wout = nc.dram_tensor("wout", [128, 2 * OUT], f32r, kind="ExternalInput").ap()
    binp = nc.dram_tensor("binp", [128, 2], f32, kind="ExternalInput").ap()
    bconv = nc.dram_tensor("bconv", [112, 2], f32, kind="ExternalInput").ap()
    boutp = nc.dram_tensor("boutp", [OUT, 1], f32, kind="ExternalInput").ap()
    out = nc.dram_tensor("out", [OUT, BS], f32, kind="ExternalOutput").ap()

    with tile.TileContext(nc) as tc, ExitStack() as ctx:
        const = ctx.enter_context(tc.tile_pool(name="const", bufs=1))
        halo_p = ctx.enter_context(tc.tile_pool(name="halo", bufs=8))
        tmp = ctx.enter_context(tc.tile_pool(name="tmp", bufs=6))
        pooled_p = ctx.enter_context(tc.tile_pool(name="pooled", bufs=1))
        rfp = ctx.enter_context(tc.tile_pool(name="rfp", bufs=4))
        outp = ctx.enter_context(tc.tile_pool(name="outp", bufs=2))
        apsum = ctx.enter_context(tc.tile_pool(name="apsum", bufs=1, space="PSUM"))

        # Warmup tiles: feed dummy matmuls during the initial DMA window so
        # the PE p-state is warm when real conv matmuls start; one relu pulls
        # the ACT function-table load forward (matters on HW, free in sim).
        t_amat = const.tile([HALO, 896], bf16)
        nc.gpsimd.dma_start(t_amat[:], amat[:])       # SWDGE: parallel to HWDGE
        warm = const.tile([128, 512], bf16, name="warm")
        nc.gpsimd.memset(warm[:], 0.0)
        warmf = const.tile([128, 16], f32, name="warmf")
        nc.gpsimd.memset(warmf[:], 0.0)
        nc.scalar.activation(warmf[:], warmf[:], RELU)

        halos = {}
        halos[0] = halo_p.tile([HALO, BS], bf16, name="halo0", tag="halo")
        nc.sync.dma_start(halos[0][:, 0:512], xt[0:112, 0:512])
        t_bconv = const.tile([112, 2], f32)
        nc.sync.dma_start(t_bconv[:], bconv[:])
        nc.sync.dma_start(halos[0][:, 512:1024], xt[0:112, 512:1024])
        for s in range(1, 3):
            halos[s] = halo_p.tile([HALO, BS], bf16, name=f"halo{s}", tag="halo")
            nc.sync.dma_start(halos[s][:], xt[56 * s:56 * s + 112, :])
        t_bin = const.tile([128, 2], f32)
        nc.sync.dma_start(t_bin[:], binp[:])
        t_wg = const.tile([112, NPOOL * H], bf16)
        nc.sync.dma_start(t_wg[:], wg[:])
        # f32r DMAs go through the Pool/SWDGE path: on the sync/HWDGE queue
        # they corrupt subsequent bf16 transfers (observed on HW).
        t_wrec = const.tile([128, 512], f32r)
        nc.gpsimd.dma_start(t_wrec[:], wrec[:])
        t_wout = const.tile([128, 2 * OUT], f32r)
        nc.gpsimd.dma_start(t_wout[:], wout[:])
        t_bout = const.tile([OUT, 1], f32)
        nc.sync.dma_start(t_bout[:], boutp[:])

        pooled = []
        for t in range(NPOOL):
            pt = pooled_p.tile([112, BS], bf16, name=f"pooled{t}", tag=f"pooled{t}")
            pooled.append(pt)

        # W_in accumulators: 4 x [128, 512] fp32, one bank per (mch, n).
        e0 = {}
        for mch in range(2):
            for n in range(2):
                e0[(mch, n)] = apsum.tile([128, 512], f32,
                                          name=f"e0_{mch}_{n}",
                                          tag=f"e0_{mch}_{n}")

        rf = {}   # (k, mch) -> [128, 1024] f32r SBUF tile
        for mch in range(2):
            rf[(0, mch)] = rfp.tile([128, BS], f32r, name=f"rf0_{mch}",
                                    tag=f"rf{mch}")

        win_done = [0, 0]   # per n-half: next t to accumulate

        def emit_win_chain(t, n):
            """W_in accumulation matmuls for pooled tile t, batch half n."""
            if win_done[n] != t:
                return   # keep accumulation order strictly ascending per half
            nsl = slice(n * 512, (n + 1) * 512)
            for mch in range(2):
                nc.tensor.matmul(
                    e0[(mch, n)][:],
                    t_wg[:, t * H + mch * 128: t * H + mch * 128 + 128],
                    pooled[t][:, nsl],
                    start=(t == 0), stop=(t == NPOOL - 1))
            win_done[n] = t + 1

        def emit_win_tail(n, t_bin):
            """Finish W_in accumulation for half n and drain r0[:, n]."""
            for t in range(win_done[n], NPOOL):
                win_done[n] = t   # satisfy the order check
                emit_win_chain(t, n)
            nsl = slice(n * 512, (n + 1) * 512)
            for mch in range(2):
                if (mch + n) % 2 == 0:
                    nc.scalar.activation(rf[(0, mch)][:, nsl],
                                         e0[(mch, n)][:], RELU,
                                         bias=t_bin[:, mch:mch + 1])
                else:
                    nc.vector.tensor_scalar(rf[(0, mch)][:, nsl],
                                            e0[(mch, n)][:],
                                            t_bin[:, mch:mch + 1], 0.0,
                                            op0=AL.add, op1=AL.max)

        # ---- conv + relu(+bias) + maxpool ----
        # gamma units: per n-half, 2 ACT relu+bias leaves + 2 DVE STT max
        # (fused bias) + 1 bf16 TT; beta units: 4 ACT leaves per half + 3 TT.
        # Mix ratio balances ACT vs DVE totals (~84us each over 28 units).
        with tc.tile_pool(name="cpsum", bufs=4, space="PSUM") as cpsum:
            # PE warmup: dummy matmuls into the first cpsum buffers while the
            # first halo DMA is in flight.
            for i in range(8):
                wp = cpsum.tile([112, 128], f32, name=f"wp{i}", tag="cv")
                nc.tensor.matmul(wp[:], warm[:, 0:112], warm[:, 0:128],
                                 start=True, stop=True)

            # Chains ci = unit*2 + n; two-stage software pipeline: stage1(ci)
            # emits the first 2 psums + ACT leaves, stage2(ci) (one chain
            # later) the last 2 psums + reduction tail. This keeps exactly 4
            # "cv" psum slots in flight and lets the ACT queue run one chain
            # ahead of the DVE tail work (beta ACT bursts stop starving DVE).
            NCHAIN = NBLK * 4
            stash = {}

            def halo_for(s):
                if s not in halos:
                    halos[s] = halo_p.tile([HALO, BS], bf16, name=f"halo{s}",
                                           tag="halo")
                    nc.sync.dma_start(halos[s][:], xt[56 * s:56 * s + 112, :])
                return halos[s]

            def stage1(ci):
                unit, n = divmod(ci, 2)
                s, j = divmod(unit, 2)
                halo = halo_for(s)
                bcj = t_bconv[:, j:j + 1]
                nsl = slice(n * 512, (n + 1) * 512)
                ps = []
                for cls in (0, 1):
                    mc = cls * 2 + j
                    p = cpsum.tile([112, 512], f32,
                                   name=f"cv{s}_{j}_{cls}_{n}", tag="cv")
                    nc.tensor.matmul(p[:], t_amat[:, mc * 112:(mc + 1) * 112],
                                     halo[:, nsl], start=True, stop=True)
                    ps.append(p)
                t0 = tmp.tile([112, 512], bf16, name=f"t0_{s}_{j}_{n}",
                              tag=f"t0_{n}")
                nc.scalar.activation(t0[:], ps[0][:], RELU, bias=bcj)
                t1 = tmp.tile([112, 512], bf16, name=f"t1_{s}_{j}_{n}",
                              tag=f"t1_{n}")
                nc.scalar.activation(t1[:], ps[1][:], RELU, bias=bcj)
                stash[ci] = (t0, t1)

            def stage2(ci):
                unit, n = divmod(ci, 2)
                s, j = divmod(unit, 2)
                halo = halos[s]
                bcj = t_bconv[:, j:j + 1]
                nsl = slice(n * 512, (n + 1) * 512)
                beta = (ci % 14) in BETA_SET
                t0, t1 = stash.pop(ci)
                ps = []
                for cls in (2, 3):
                    mc = cls * 2 + j
                    p = cpsum.tile([112, 512], f32,
                                   name=f"cv{s}_{j}_{cls}_{n}", tag="cv")
                    nc.tensor.matmul(p[:], t_amat[:, mc * 112:(mc + 1) * 112],
                                     halo[:, nsl], start=True, stop=True)
                    ps.append(p)
                if not beta:
                    m1 = tmp.tile([112, 512], bf16, name=f"m1_{s}_{j}_{n}",
                                  tag=f"m1_{n}")
                    nc.vector.scalar_tensor_tensor(
                        m1[:], ps[0][:], bcj, t0[:], op0=AL.add, op1=AL.max)
                    m2 = tmp.tile([112, 512], bf16, name=f"m2_{s}_{j}_{n}",
                                  tag=f"m2_{n}")
                    nc.vector.scalar_tensor_tensor(
                        m2[:], ps[1][:], bcj, t1[:], op0=AL.add, op1=AL.max)
                    nc.vector.tensor_max(pooled[unit][:, nsl], m1[:], m2[:])
                else:
                    m1 = tmp.tile([112, 512], bf16, name=f"m1_{s}_{j}_{n}",
                                  tag=f"m1_{n}")
                    nc.scalar.activation(m1[:], ps[0][:], RELU, bias=bcj)
                    m2 = tmp.tile([112, 512], bf16, name=f"m2_{s}_{j}_{n}",
                                  tag=f"m2_{n}")
                    nc.scalar.activation(m2[:], ps[1][:], RELU, bias=bcj)
                    u1 = tmp.tile([112, 512], bf16, name=f"u1_{s}_{j}_{n}",
                                  tag=f"u1_{n}")
                    nc.vector.tensor_max(u1[:], t0[:], t1[:])
                    stash[("b", ci)] = (m1, m2, u1)

            def stage3(ci):
                # deferred beta-chain combines: lets the DVE run the next
                # chain's STTs while the beta ACT leaves finish
                if ("b", ci) not in stash:
                    return
                unit, n = divmod(ci, 2)
                s, j = divmod(unit, 2)
                nsl = slice(n * 512, (n + 1) * 512)
                m1, m2, u1 = stash.pop(("b", ci))
                u2 = tmp.tile([112, 512], bf16, name=f"u2_{s}_{j}_{n}",
                              tag=f"u2_{n}")
                nc.vector.tensor_max(u2[:], m1[:], m2[:])
                nc.vector.tensor_max(pooled[unit][:, nsl], u1[:], u2[:])

            SK = WSKEW
            for ci in range(NCHAIN + 2):
                if ci < NCHAIN:
                    stage1(ci)
                if 2 <= ci:
                    stage3(ci - 2)
                if 1 <= ci <= NCHAIN:
                    stage2(ci - 1)
                    if SK <= ci - 1 < NCHAIN - 2:
                        cw = ci - 1 - SK
                        emit_win_chain(cw // 2, cw % 2)
                if ci == NCHAIN:          # chains 0..54 fully drained
                    emit_win_tail(0, t_bin)
                elif ci == NCHAIN + 1:    # all chains drained
                    emit_win_tail(1, t_bin)

        # ---- 9 recurrent steps: r' = relu(r @ (W+I) + b_in), f32r PE ----
        # All rec psums reuse the freed e0 rings (apsum), so no separate psum
        # pool scope is needed and the n=0 chains overlap the conv tail.
        NQ = REC_NQ
        QW = BS // NQ
        if True:
            def pr_tile(k, mch, q):
                return apsum.tile([128, QW], f32, name=f"pr{k}_{mch}_{q}",
                                  tag=f"e0_{mch}_{q % 2 if NQ == 2 else q % 2}")

            for k in range(1, NSTEP + 1):
                for mch in range(2):
                    rf[(k, mch)] = rfp.tile([128, BS], f32r, name=f"rf{k}_{mch}",
                                            tag=f"rf{mch}")
                for q in range(NQ):
                    nq = slice(q * QW, (q + 1) * QW)
                    prs = []
                    for mch in range(2):
                        pr = pr_tile(k, mch, q)
                        for kc in range(2):
                            nc.tensor.matmul(
                                pr[:],
                                t_wrec[:, (kc * 2 + mch) * 128:(kc * 2 + mch) * 128 + 128],
                                rf[(k - 1, kc)][:, nq],
                                start=(kc == 0), stop=(kc == 1))
                        prs.append(pr)
                    for mch in range(2):
                        bia = t_bin[:, mch:mch + 1]
                        if (k + q + mch) % 2 == 0:
                            nc.scalar.activation(rf[(k, mch)][:, nq], prs[mch][:],
                                                 RELU, bias=bia)
                        else:
                            nc.vector.tensor_scalar(rf[(k, mch)][:, nq],
                                                    prs[mch][:], bia, 0.0,
                                                    op0=AL.add, op1=AL.max)
                    if k == NSTEP and (q + 1) * QW % 512 == 0:
                        # ---- W_out + b_out for the finished batch half ----
                        n = ((q + 1) * QW) // 512 - 1
                        nsl = slice(n * 512, (n + 1) * 512)
                        po = apsum.tile([OUT, 512], f32, name=f"po{n}",
                                        tag=f"e0_1_{n}")
                        for kc in range(2):
                            nc.tensor.matmul(
                                po[:], t_wout[:, kc * OUT:(kc + 1) * OUT],
                                rf[(NSTEP, kc)][:, nsl],
                                start=(kc == 0), stop=(kc == 1))
                        ot = outp.tile([OUT, 512], f32, name=f"ot{n}", tag="ot")
                        nc.vector.tensor_scalar_add(ot[:], po[:], t_bout[:, 0:1])
                        nc.sync.dma_start(out[:, nsl], ot[:])

    nc.compile()
    return nc


def _prep_host(inputs):
    x = np.asarray(inputs["x"], np.float32).reshape(B, 784)
    conv_w = np.asarray(inputs["conv_w"], np.float32)
    conv_b = np.asarray(inputs["conv_b"], np.float32)
    W_in = np.asarray(inputs["W_in"], np.float32)
    b_in = np.asarray(inputs["b_in"], np.float32)
    W_out = np.asarray(inputs["W_out"], np.float32)
    b_out = np.asarray(inputs["b_out"], np.float32)
    W = np.asarray(inputs["W"], np.float32)

    xT = np.zeros((840, B), np.float32)
    xT[28:812, :] = x.T
    xT = xT.astype(BF16)

    A = _build_amat(conv_w).astype(BF16)
    q = np.arange(112)
    bconv = np.stack([conv_b[8 * j + q // 14] for j in range(2)], axis=1).astype(np.float32)
    Wg = _build_wg(W_in).astype(BF16)

    # wrec [128, 4*128]: blocks of (W + I), consumed as float32r by the PE.
    WI = W + np.eye(H, dtype=np.float32)
    wrec = np.zeros((128, 512), np.float32)
    for kc in range(2):
        for mch in range(2):
            wrec[:, (kc * 2 + mch) * 128:(kc * 2 + mch) * 128 + 128] = \
                WI[kc * 128:(kc + 1) * 128, mch * 128:(mch + 1) * 128]

    woutb = np.zeros((128, 2 * OUT), np.float32)
    for kc in range(2):
        woutb[:, kc * OUT:(kc + 1) * OUT] = W_out[:, kc * 128:(kc + 1) * 128].T

    binp = b_in.reshape(2, 128).T.copy()
    boutp = b_out.reshape(OUT, 1).astype(np.float32)

    common = {"amat": A, "wg": Wg, "wrec": wrec, "wout": woutb,
              "binp": binp, "boutp": boutp, "bconv": bconv}
    in_maps = []
    for c in range(NCORES):
        m = dict(common)
        m["xt"] = np.ascontiguousarray(xT[:, c * BS:(c + 1) * BS])
        in_maps.append(m)
    return in_maps


def kernel(**inputs):
    if "nc" not in _CACHE:
        _CACHE["nc"] = _build_graph()
    nc = _CACHE["nc"]
    in_maps = _prep_host(inputs)
    res = run_bass_kernel_spmd(nc, in_maps, core_ids=list(range(NCORES)))
    _CACHE["last_result"] = res
    outs = [res.results[c]["out"].T for c in range(NCORES)]
    return np.ascontiguousarray(np.concatenate(outs, axis=0)).astype(np.float32)
